# revision 10
# baseline (speedup 1.0000x reference)
"""Trainium2 Bass kernel for AttentionBlock3D (GroupNorm + spatial attention per
depth slice + 1x1 conv proj + residual).

Sharding: the 16 (b,d) slices are distributed 2-per-core across 8 NeuronCores
(cores 0-3 -> batch 0, cores 4-7 -> batch 1). GroupNorm statistics couple all
depth slices of a batch, so each core computes partial (sum, sumsq) per group
over its 2 slices and a tiny AllReduce within each batch's 4-core group
produces the full stats on-device.

Attention math per (b,d) slice, with N = H*W = 2304 spatial positions:
  S^T[HW, hw] = sum_c k[c,HW] q[c,hw]        (keys on partitions -> AV needs no
                                              transpose of the attention matrix)
  softmax over the key W axis == partition-blocks of 48 in S^T; block sums are
  computed with a block-diagonal ones matmul on the tensor engine, expanded
  back with its transpose, and applied with one vector multiply.
  av[c,hw]   = sum_HW V^T[HW,c] A^T[HW,hw]
V^T is produced directly from the qkv matmul by swapping the matmul orientation.
GroupNorm is folded into the qkv weights (per-channel scale/bias on the
contraction dim), and the V-path bias is applied post-attention using
sum_HW A^T[HW,hw] == 48 exactly.

The attention/projection matmuls run in bf16 (separate LDWEIGHTS instructions
that pipeline with matmuls, unlike the self-loading 4-byte path); statistics
and the softmax denominator reciprocal stay in f32.
"""

import numpy as np

import concourse.bass as bass
import concourse.bacc as bacc
import concourse.mybir as mybir
import concourse.tile as tile
from concourse import bass_utils

F32 = mybir.dt.float32
BF16 = mybir.dt.bfloat16
AF = mybir.ActivationFunctionType
ALU = mybir.AluOpType
AX = mybir.AxisListType

B, C, D, H, W = 2, 256, 8, 48, 48
N = H * W                      # 2304 spatial positions per depth slice
G = 32                         # groups
CPG = C // G                   # channels per group
CNT = CPG * D * H * W          # elements per (batch, group) for the stats
EPS = 1e-5
SCALE_Q = float((C // 8) ** (-0.5))
NCORES = 8
NT = N // 128                  # 18 key tiles of 128
JCH = [(0, 512), (512, 512), (1024, 512), (1536, 512), (2048, 256)]

_CACHE: dict = {}


def _build_nc():
    nc = bacc.Bacc(trn_type="TRN2", num_devices=NCORES)

    xs_d = nc.dram_tensor("xs", [2, C, N], F32, kind="ExternalInput")
    qkvwT_d = nc.dram_tensor("qkv_wT", [C, 3 * C], F32, kind="ExternalInput")
    qkvb_d = nc.dram_tensor("qkv_b2", [1, 3 * C], F32, kind="ExternalInput")
    projwT_d = nc.dram_tensor("proj_wT", [C, C], BF16, kind="ExternalInput")
    projb_d = nc.dram_tensor("proj_b2", [C, 1], F32, kind="ExternalInput")
    nw_d = nc.dram_tensor("norm_w2", [C, 1], F32, kind="ExternalInput")
    nb_d = nc.dram_tensor("norm_b2", [C, 1], F32, kind="ExternalInput")
    sel_d = nc.dram_tensor("sel", [C, G], F32, kind="ExternalInput")
    selT_d = nc.dram_tensor("selT", [G, C], F32, kind="ExternalInput")
    bo_d = nc.dram_tensor("bo", [N, H], BF16, kind="ExternalInput")
    boT_d = nc.dram_tensor("boT", [H, N], BF16, kind="ExternalInput")
    out_d = nc.dram_tensor("out", [2, C, N], F32, kind="ExternalOutput")
    cc_in = nc.dram_tensor("cc_in", [G, 2], F32)
    cc_out = nc.dram_tensor("cc_out", [G, 2], F32)

    with tile.TileContext(nc) as tc:
        with (
            tc.tile_pool(name="cst", bufs=1) as cst,
            tc.tile_pool(name="big", bufs=1) as big,
            tc.tile_pool(name="scp", bufs=2) as scp,
            tc.tile_pool(name="psp", bufs=3, space="PSUM") as psp,
            tc.tile_pool(name="pavp", bufs=2, space="PSUM") as pavp,
            tc.tile_pool(name="pzp", bufs=1, space="PSUM") as pzp,
        ):
            # ---------------- constant / weight loads ----------------
            bo_sb, boT_sb = [], []
            for m in range(NT):
                t1 = cst.tile([128, H], BF16, name=f"bo{m}", tag=f"bo{m}")
                nc.sync.dma_start(out=t1, in_=bo_d[128 * m:128 * (m + 1), :])
                bo_sb.append(t1)
                t2 = cst.tile([H, 128], BF16, name=f"boT{m}", tag=f"boT{m}")
                nc.sync.dma_start(out=t2, in_=boT_d[:, 128 * m:128 * (m + 1)])
                boT_sb.append(t2)

            wraw, wf, pw, sel_sb, nw_sb, nb_sb, pb_sb = [], [], [], [], [], [], []
            for cc in range(2):
                r0, r1 = 128 * cc, 128 * (cc + 1)
                t = cst.tile([128, 3 * C], F32, name=f"wraw{cc}", tag=f"wraw{cc}")
                nc.sync.dma_start(out=t, in_=qkvwT_d[r0:r1, :])
                wraw.append(t)
                wf.append(cst.tile([128, 3 * C], BF16, name=f"wf{cc}", tag=f"wf{cc}"))
                t = cst.tile([128, C], BF16, name=f"pw{cc}", tag=f"pw{cc}")
                nc.sync.dma_start(out=t, in_=projwT_d[r0:r1, :])
                pw.append(t)
                t = cst.tile([128, G], F32, name=f"selc{cc}", tag=f"selc{cc}")
                nc.sync.dma_start(out=t, in_=sel_d[r0:r1, :])
                sel_sb.append(t)
                t = cst.tile([128, 1], F32, name=f"nw{cc}", tag=f"nw{cc}")
                nc.sync.dma_start(out=t, in_=nw_d[r0:r1, :])
                nw_sb.append(t)
                t = cst.tile([128, 1], F32, name=f"nb{cc}", tag=f"nb{cc}")
                nc.sync.dma_start(out=t, in_=nb_d[r0:r1, :])
                nb_sb.append(t)
                t = cst.tile([128, 1], F32, name=f"pb{cc}", tag=f"pb{cc}")
                nc.sync.dma_start(out=t, in_=projb_d[r0:r1, :])
                pb_sb.append(t)
            selT_sb = cst.tile([G, C], F32, name="selT", tag="selT")
            nc.sync.dma_start(out=selT_sb, in_=selT_d[:, :])
            qkvb_sb = cst.tile([1, 3 * C], F32, name="qkvb", tag="qkvb")
            nc.sync.dma_start(out=qkvb_sb, in_=qkvb_d[:, :])

            X = [[None, None], [None, None]]
            Xb = [[None, None], [None, None]]
            for s in range(2):
                for cc in range(2):
                    t = big.tile([128, N], F32, name=f"x{s}{cc}", tag=f"x{s}{cc}")
                    nc.sync.dma_start(
                        out=t, in_=xs_d[s, 128 * cc:128 * (cc + 1), :]
                    )
                    X[s][cc] = t
                    tb = big.tile([128, N], BF16, name=f"xb{s}{cc}", tag=f"xb{s}{cc}")
                    nc.scalar.activation(tb, t, AF.Copy, bias=0.0, scale=1.0)
                    Xb[s][cc] = tb

            # ---------------- group-norm statistics ----------------
            with nc.named_scope("stats"):
                stats = []
                for cc in range(2):
                    stats.append(
                        cst.tile([128, 4], F32, name=f"stat{cc}", tag=f"stat{cc}")
                    )
                for s in range(2):
                    for cc in range(2):
                        ssum = cst.tile([128, 1], F32, name=f"ssum{s}{cc}", tag=f"ssum{s}{cc}")
                        nc.vector.reduce_sum(out=ssum, in_=X[s][cc], axis=AX.X)
                        sq = scp.tile([128, N], F32, name="sq", tag="sq", bufs=2)
                        ssq = cst.tile([128, 1], F32, name=f"ssq{s}{cc}", tag=f"ssq{s}{cc}")
                        nc.scalar.activation(
                            sq, X[s][cc], AF.Square, bias=0.0, scale=1.0,
                            accum_out=ssq,
                        )
                        nc.vector.tensor_copy(out=stats[cc][:, 2 * s:2 * s + 1], in_=ssum)
                        nc.vector.tensor_copy(out=stats[cc][:, 2 * s + 1:2 * s + 2], in_=ssq)
                pst = pzp.tile([G, 4], F32, name="pst", tag="pst")
                nc.tensor.matmul(out=pst, lhsT=sel_sb[0], rhs=stats[0], start=True, stop=False)
                nc.tensor.matmul(out=pst, lhsT=sel_sb[1], rhs=stats[1], start=False, stop=True)
                st4 = cst.tile([G, 4], F32, name="st4", tag="st4")
                nc.vector.tensor_copy(out=st4, in_=pst)
                cc_sb = cst.tile([G, 2], F32, name="ccsb", tag="ccsb")
                nc.vector.tensor_add(cc_sb, st4[:, 0:2], st4[:, 2:4])
                nc.sync.dma_start(out=cc_in[:, :], in_=cc_sb)
                nc.gpsimd.collective_compute(
                    "AllReduce",
                    ALU.add,
                    replica_groups=[[0, 1, 2, 3], [4, 5, 6, 7]],
                    ins=[cc_in.ap().opt()],
                    outs=[cc_out.ap().opt()],
                )
                red = cst.tile([G, 2], F32, name="red", tag="red")
                nc.sync.dma_start(out=red, in_=cc_out[:, :])

            with nc.named_scope("fold"):
                mu = cst.tile([G, 1], F32, name="mu", tag="mu")
                nc.vector.tensor_scalar_mul(mu, red[:, 0:1], 1.0 / CNT)
                ex2 = cst.tile([G, 1], F32, name="ex2", tag="ex2")
                nc.vector.tensor_scalar_mul(ex2, red[:, 1:2], 1.0 / CNT)
                mu2 = cst.tile([G, 1], F32, name="mu2", tag="mu2")
                nc.vector.tensor_mul(mu2, mu, mu)
                var = cst.tile([G, 1], F32, name="var", tag="var")
                nc.vector.tensor_sub(var, ex2, mu2)
                epst = cst.tile([G, 1], F32, name="epst", tag="epst")
                nc.vector.memset(epst, EPS)
                sd = cst.tile([G, 1], F32, name="sd", tag="sd")
                nc.scalar.activation(sd, var, AF.Sqrt, bias=epst, scale=1.0)
                rm = cst.tile([G, 2], F32, name="rm", tag="rm")
                nc.vector.reciprocal(out=rm[:, 0:1], in_=sd)
                nc.vector.tensor_mul(rm[:, 1:2], mu, rm[:, 0:1])

                # expand group stats to per-channel scale/bias, fold into weights
                nscale, nsq, nbias = [], [], []
                for cc in range(2):
                    pse = pzp.tile([128, 2], F32, name="pse", tag="pst")
                    nc.tensor.matmul(
                        out=pse, lhsT=selT_sb[:, 128 * cc:128 * (cc + 1)], rhs=rm,
                        start=True, stop=True,
                    )
                    e = cst.tile([128, 2], F32, name=f"e{cc}", tag=f"e{cc}")
                    nc.vector.tensor_copy(out=e, in_=pse)
                    nsc = cst.tile([128, 1], F32, name=f"nsc{cc}", tag=f"nsc{cc}")
                    nc.vector.tensor_mul(nsc, e[:, 0:1], nw_sb[cc])
                    nscale.append(nsc)
                    nscq = cst.tile([128, 1], F32, name=f"nscq{cc}", tag=f"nscq{cc}")
                    nc.vector.tensor_scalar_mul(nscq, nsc, SCALE_Q)
                    nsq.append(nscq)
                    tmp = cst.tile([128, 1], F32, name=f"tmp{cc}", tag=f"tmp{cc}")
                    nc.vector.tensor_mul(tmp, e[:, 1:2], nw_sb[cc])
                    nbi = cst.tile([128, 1], F32, name=f"nbi{cc}", tag=f"nbi{cc}")
                    nc.vector.tensor_sub(nbi, nb_sb[cc], tmp)
                    nbias.append(nbi)
                    nc.vector.tensor_scalar_mul(wf[cc][:, 0:C], wraw[cc][:, 0:C], nscq)
                    nc.vector.tensor_scalar_mul(
                        wf[cc][:, C:3 * C], wraw[cc][:, C:3 * C], nsc
                    )

                ones11 = cst.tile([1, 1], F32, name="ones11", tag="ones11")
                nc.vector.memset(ones11, 1.0)
                tvec = []
                for ot in range(6):
                    o0, o1 = 128 * ot, 128 * (ot + 1)
                    ptt = pzp.tile([128, 1], F32, name="ptt", tag="pst")
                    nc.tensor.matmul(out=ptt, lhsT=wraw[0][:, o0:o1], rhs=nbias[0],
                                     start=True, stop=False)
                    nc.tensor.matmul(out=ptt, lhsT=wraw[1][:, o0:o1], rhs=nbias[1],
                                     start=False, stop=False)
                    nc.tensor.matmul(out=ptt, lhsT=qkvb_sb[0:1, o0:o1], rhs=ones11,
                                     start=False, stop=True)
                    t = cst.tile([128, 1], F32, name=f"t{ot}", tag=f"t{ot}")
                    nc.scalar.activation(
                        t, ptt, AF.Identity, bias=0.0,
                        scale=SCALE_Q if ot < 2 else 1.0,
                    )
                    tvec.append(t)
                av_bias = []
                for i in range(2):
                    t = cst.tile([128, 1], F32, name=f"avb{i}", tag=f"avb{i}")
                    nc.vector.tensor_scalar_mul(t, tvec[4 + i], float(H))
                    av_bias.append(t)

            # ---------------- per-slice attention ----------------
            for s in range(2):
                with nc.named_scope(f"qkv{s}"):
                    q, k = [], []
                    for cc in range(2):
                        q.append(big.tile([128, N], BF16, name=f"q{cc}", tag=f"q{cc}"))
                        k.append(big.tile([128, N], BF16, name=f"k{cc}", tag=f"k{cc}"))
                    for ot in range(4):
                        dest = q[ot] if ot < 2 else k[ot - 2]
                        for (j0, jw) in JCH:
                            pqk = psp.tile([128, jw], F32, name="pqk", tag="ps")
                            nc.tensor.matmul(
                                out=pqk,
                                lhsT=wf[0][:, 128 * ot:128 * (ot + 1)],
                                rhs=Xb[s][0][:, j0:j0 + jw],
                                start=True, stop=False,
                            )
                            nc.tensor.matmul(
                                out=pqk,
                                lhsT=wf[1][:, 128 * ot:128 * (ot + 1)],
                                rhs=Xb[s][1][:, j0:j0 + jw],
                                start=False, stop=True,
                            )
                            nc.scalar.activation(
                                dest[:, j0:j0 + jw], pqk, AF.Identity,
                                bias=tvec[ot], scale=1.0,
                            )
                    vt = []
                    for m in range(NT):
                        n0, n1 = 128 * m, 128 * (m + 1)
                        pv = psp.tile([128, C], F32, name="pv", tag="ps")
                        nc.tensor.matmul(
                            out=pv, lhsT=Xb[s][0][:, n0:n1],
                            rhs=wf[0][:, 2 * C:3 * C], start=True, stop=False,
                        )
                        nc.tensor.matmul(
                            out=pv, lhsT=Xb[s][1][:, n0:n1],
                            rhs=wf[1][:, 2 * C:3 * C], start=False, stop=True,
                        )
                        v = big.tile([128, C], BF16, name="vt", tag="vt", bufs=NT)
                        nc.vector.tensor_copy(out=v, in_=pv)
                        vt.append(v)

                with nc.named_scope(f"attn{s}"):
                    for (j0, jw) in JCH:
                        # pass A: S^T tiles -> exp, with Z matmuls interleaved
                        ets = []
                        pz = pzp.tile([H, jw], F32, name="pz", tag="pz")
                        for m in range(NT):
                            n0, n1 = 128 * m, 128 * (m + 1)
                            pss = psp.tile([128, jw], F32, name="pss", tag="ps")
                            nc.tensor.matmul(
                                out=pss, lhsT=k[0][:, n0:n1],
                                rhs=q[0][:, j0:j0 + jw], start=True, stop=False,
                            )
                            nc.tensor.matmul(
                                out=pss, lhsT=k[1][:, n0:n1],
                                rhs=q[1][:, j0:j0 + jw], start=False, stop=True,
                            )
                            et = big.tile([128, jw], BF16, name="et", tag="et", bufs=NT)
                            nc.scalar.activation(et, pss, AF.Exp, bias=0.0, scale=1.0)
                            ets.append(et)
                            if m >= 1:
                                nc.tensor.matmul(
                                    out=pz, lhsT=bo_sb[m - 1], rhs=ets[m - 1],
                                    start=(m == 1), stop=False,
                                )
                        nc.tensor.matmul(
                            out=pz, lhsT=bo_sb[NT - 1], rhs=ets[NT - 1],
                            start=False, stop=True,
                        )
                        zr = scp.tile([H, jw], F32, name="zr", tag="zr", bufs=2)
                        nc.vector.reciprocal_approx_fast(out=zr, in_=pz)
                        zrb = scp.tile([H, jw], BF16, name="zrb", tag="zrb", bufs=2)
                        nc.vector.tensor_copy(out=zrb, in_=zr)

                        # pass B: expand 1/Z, normalize, AV matmuls (pipelined)
                        pav = [
                            pavp.tile([128, jw], F32, name=f"pav{ct}", tag="pav")
                            for ct in range(2)
                        ]
                        prs = [psp.tile([128, jw], F32, name="pr", tag="ps")]
                        nc.tensor.matmul(
                            out=prs[0], lhsT=boT_sb[0], rhs=zrb,
                            start=True, stop=True,
                        )
                        for m in range(NT):
                            if m < NT - 1:
                                pr_next = psp.tile([128, jw], F32, name="pr", tag="ps")
                                nc.tensor.matmul(
                                    out=pr_next, lhsT=boT_sb[m + 1], rhs=zrb,
                                    start=True, stop=True,
                                )
                                prs.append(pr_next)
                            nc.vector.tensor_mul(ets[m], ets[m], prs[m])
                            for ct in range(2):
                                nc.tensor.matmul(
                                    out=pav[ct],
                                    lhsT=vt[m][:, 128 * ct:128 * (ct + 1)],
                                    rhs=ets[m],
                                    start=(m == 0), stop=(m == NT - 1),
                                )
                        av_sb = []
                        for ct in range(2):
                            a = scp.tile([128, jw], BF16, name="avsb", tag="avsb", bufs=4)
                            nc.scalar.activation(
                                a, pav[ct], AF.Identity, bias=av_bias[ct], scale=1.0
                            )
                            av_sb.append(a)
                        for ot in range(2):
                            pp = psp.tile([128, jw], F32, name="pp", tag="ps")
                            nc.tensor.matmul(
                                out=pp, lhsT=pw[0][:, 128 * ot:128 * (ot + 1)],
                                rhs=av_sb[0], start=True, stop=False,
                            )
                            nc.tensor.matmul(
                                out=pp, lhsT=pw[1][:, 128 * ot:128 * (ot + 1)],
                                rhs=av_sb[1], start=False, stop=True,
                            )
                            osb = scp.tile([128, jw], F32, name="osb", tag="osb", bufs=4)
                            nc.vector.scalar_tensor_tensor(
                                out=osb, in0=pp, scalar=pb_sb[ot],
                                in1=X[s][ot][:, j0:j0 + jw],
                                op0=ALU.add, op1=ALU.add,
                            )
                            nc.sync.dma_start(
                                out=out_d[s, 128 * ot:128 * (ot + 1), j0:j0 + jw],
                                in_=osb,
                            )
    nc.finalize()
    return nc


def _consts():
    sel = np.zeros((C, G), np.float32)
    sel[np.arange(C), np.arange(C) // CPG] = 1.0
    bo = np.zeros((N, H), np.float32)
    bo[np.arange(N), np.arange(N) // W] = 1.0
    return sel, bo


def _prep_inputs(x, norm_w, norm_b, qkv_w, qkv_b, proj_w, proj_b):
    import ml_dtypes
    bf16 = ml_dtypes.bfloat16
    x = np.ascontiguousarray(np.asarray(x, np.float32))
    norm_w = np.asarray(norm_w, np.float32).reshape(C, 1)
    norm_b = np.asarray(norm_b, np.float32).reshape(C, 1)
    qkv_wT = np.ascontiguousarray(np.asarray(qkv_w, np.float32).T)
    qkv_b2 = np.asarray(qkv_b, np.float32).reshape(1, 3 * C)
    proj_wT = np.ascontiguousarray(np.asarray(proj_w, np.float32).T.astype(bf16))
    proj_b2 = np.asarray(proj_b, np.float32).reshape(C, 1)
    sel, bo = _consts()
    shared = {
        "qkv_wT": qkv_wT, "qkv_b2": qkv_b2, "proj_wT": proj_wT,
        "proj_b2": proj_b2, "norm_w2": norm_w, "norm_b2": norm_b,
        "sel": sel, "selT": np.ascontiguousarray(sel.T),
        "bo": bo.astype(bf16), "boT": np.ascontiguousarray(bo.T).astype(bf16),
    }
    in_maps = []
    for kk in range(NCORES):
        b, d0 = kk // 4, 2 * (kk % 4)
        xs = np.ascontiguousarray(
            x[b, :, d0:d0 + 2].transpose(1, 0, 2, 3).reshape(2, C, N)
        )
        in_maps.append({"xs": xs, **shared})
    return in_maps


def run(inputs: dict, trace: bool = False):
    if "nc" not in _CACHE:
        _CACHE["nc"] = _build_nc()
    nc = _CACHE["nc"]
    in_maps = _prep_inputs(**inputs)
    res = bass_utils.run_bass_kernel_spmd(
        nc, in_maps, core_ids=list(range(NCORES)), trace=trace
    )
    y = np.empty((B, C, D, H, W), np.float32)
    for kk in range(NCORES):
        b, d0 = kk // 4, 2 * (kk % 4)
        o = res.results[kk]["out"].reshape(2, C, H, W)
        y[b, :, d0] = o[0]
        y[b, :, d0 + 1] = o[1]
    return y, res


def kernel(**inputs) -> np.ndarray:
    y, _ = run(inputs, trace=False)
    return y
